# revision 6
# baseline (speedup 1.0000x reference)
"""Trainium2 Bass kernel for nn_CrossAttention_5385888989393.

Contract: kernel(**inputs) takes FULL inputs (batch 8) and returns the FULL
output, sharding batch-parallel across 8 NeuronCores (1 batch element per
core, no collectives).

Algorithm per batch (channel attention, contraction over spatial n=4096):
    G     = f_m @ f_n^T                     [512, 512]  Gram over n
    T2T   = G^T @ Wq^T                      [512, 512]  (G stationary)
    D^T_h = Wk_h-contraction with T2T       [64, 64] per head (diag tiles)
    E^T   = exp(D^T * scale) * headmask     (softmax numerator, transposed)
    SE_h  = E_h @ Wv_h   (via lhsT = E^T)   [64, 512]
    S_h   = SE_h / rowsum(E_h)              (deferred softmax normalization)
    M^T   = S-contraction with Wout^T       [512, 512]
    out   = (M @ f_n) + bout                [512, 4096]

~2x fewer FLOPs than the naive q/k/v path (spatial dim collapses through
the Gram immediately).

Perf notes (from NTFF traces of prior versions):
 - bf16 staging everywhere: enables the PE's automatic Fast Weight Load
   (fp32r stationaries disable FWL, serializing ~170ns LDWEIGHTS behind
   every matmul: measured 240ns/transpose fp32r vs ~95ns bf16), halves
   DVE copy cost, lifts the fp32r small-free-dim penalty. PSUM stays fp32.
 - DMA dispatch costs ~620ns of SP-sequencer time per dma_start; at 119
   DMAs that was ~73us of Sync, co-critical with PE. So: inputs load in
   [128,1024] chunk-pairs (after two [128,512] warm-start chunks), each
   weight matrix is one [p,(t c)] DMA, all small constants ship as one
   packed [128,260] array, outputs store as [128,1024] pairs: ~61 DMAs.
 - Warm-up transposes source a gpsimd-memset tile (no DMA dependency) so
   the PE p-state ramp starts at ~3us instead of ~10us.
 - Conversions split across ACT/DVE/GpSimd (GpSimd is otherwise idle).
"""
import sys

if "/opt/trn_rl_repo" not in sys.path:
    sys.path.insert(0, "/opt/trn_rl_repo")

import numpy as np

import concourse.bass as bass
import concourse.tile as tile
from concourse import bacc, mybir
from concourse.bass_utils import run_bass_kernel_spmd
F32 = mybir.dt.float32
BF16 = mybir.dt.bfloat16
EXP = mybir.ActivationFunctionType.Exp
CP = mybir.ActivationFunctionType.Copy
IDENT_FN = mybir.ActivationFunctionType.Identity

P = 128          # partitions
C = 512          # channels
CT = C // P      # 4 channel tiles
NN = 4096        # spatial (64*64)
NCH = NN // 512  # 8 column chunks of 512
NSUB = NN // P   # 32 column subchunks of 128
DH = 64
SCALE = DH ** -0.5
B = 8            # batch == n_cores

# input chunk groups: two single-width warm-start groups, then pairs
GROUPS = [(0,), (1,), (2, 3), (4, 5), (6, 7)]

_CACHED_NC = None
_CACHED_RUNNER = None

_IDENT = np.eye(P, dtype=np.float32)
_DMASK = np.kron(np.eye(2, dtype=np.float32), np.ones((DH, DH), np.float32))


def _build():
    nc = bacc.Bacc("TRN2", target_bir_lowering=False, debug=False, num_devices=B)

    fm_d = nc.dram_tensor("f_m", [C, NN], F32, kind="ExternalInput").ap()
    fn_d = nc.dram_tensor("f_n", [C, NN], F32, kind="ExternalInput").ap()
    wqt_d = nc.dram_tensor("WqT", [C, C], F32, kind="ExternalInput").ap()
    wkt_d = nc.dram_tensor("WkT", [C, C], F32, kind="ExternalInput").ap()
    wv_d = nc.dram_tensor("Wv", [C, C], F32, kind="ExternalInput").ap()
    woutt_d = nc.dram_tensor("WoutT", [C, C], F32, kind="ExternalInput").ap()
    # packed [128, 260]: identity | head-mask | bout as 4 columns
    cst_d = nc.dram_tensor("cst", [P, 260], F32, kind="ExternalInput").ap()
    out_d = nc.dram_tensor("out", [C, NN], F32, kind="ExternalOutput").ap()

    with tile.TileContext(nc) as tc:
        with (
            tc.tile_pool(name="const", bufs=1) as const,
            tc.tile_pool(name="w", bufs=1) as wpool,
            tc.tile_pool(name="wstage", bufs=2) as wstage,
            tc.tile_pool(name="fnst", bufs=2) as fnst,
            tc.tile_pool(name="fmst", bufs=2) as fmst,
            tc.tile_pool(name="fmb", bufs=2) as fmbpool,
            tc.tile_pool(name="ft", bufs=3) as ftpool,
            tc.tile_pool(name="small", bufs=1) as small,
            tc.tile_pool(name="fnb", bufs=1) as fnbpool,
            tc.tile_pool(name="outst", bufs=2) as outst,
            tc.tile_pool(name="gacc", bufs=1, space="PSUM") as gacc,
            tc.tile_pool(name="work", bufs=2, space="PSUM") as work,
        ):
            # ---------- DMA-free warm-up ------------------------------------
            # HAM warm-up: back-to-back transposes of a memset tile fill the
            # otherwise PE-idle startup window (waiting on the first data
            # chunk) with sustained PE activity, so the first real matmuls
            # run at 2.4 GHz instead of the cold 1.2 GHz. Sourcing from a
            # gpsimd memset (not a DMA) lets the ramp start at ~3us. The
            # written values are garbage and never read; ordering vs the
            # first data transposes comes from the WAW dep on the wk0 tile.
            warmsrc = const.tile([P, P], BF16, tag="warmsrc")
            nc.vector.memset(warmsrc[:], 1.0)
            warm_ps = work.tile([P, C], BF16, tag="wk0", name="warmps")
            for i in range(44):
                wsl = slice((i % 4) * P, ((i % 4) + 1) * P)
                nc.tensor.transpose(warm_ps[:, wsl], warmsrc[:], warmsrc[:])

            # ones rhs for the rowsum matmuls (exact in bf16)
            ones2_b = const.tile([P, 2], BF16, tag="ones2_b")
            nc.vector.memset(ones2_b[:], 1.0)

            # ---------- packed constants ------------------------------------
            cst = const.tile([P, 260], F32, tag="cst")
            nc.sync.dma_start(cst[:], cst_d)
            identb = const.tile([P, P], BF16, tag="identb")
            nc.vector.tensor_copy(identb[:], cst[:, 0:P])
            # block-diagonal 0/1 mask to zero cross-head blocks of E^T
            dmask = const.tile([P, P], BF16, tag="dmask")
            nc.vector.tensor_copy(dmask[:], cst[:, P:2 * P])
            bout_sb = [cst[:, 2 * P + ct:2 * P + ct + 1] for ct in range(CT)]

            # ---------- phase 1: Gram accumulation over 32 subchunks --------
            g_ps = [
                gacc.tile([P, C], F32, tag=f"g{at}", name=f"g_ps{at}")
                for at in range(CT)
            ]

            fnb_by_ch = [[None] * NCH for _ in range(CT)]
            for gi, grp in enumerate(GROUPS):
                w = 512 * len(grp)
                col0 = grp[0] * 512
                fm_tiles = {}
                fn_tiles = {}
                for ct in range(CT):
                    t = fmst.tile([P, 1024], F32, tag=f"fmst{ct}")
                    nc.sync.dma_start(
                        t[:, 0:w], fm_d[ct * P:(ct + 1) * P, col0:col0 + w]
                    )
                    fm_tiles[ct] = t
                    t = fnst.tile([P, 1024], F32, tag=f"fnst{ct}")
                    nc.sync.dma_start(
                        t[:, 0:w], fn_d[ct * P:(ct + 1) * P, col0:col0 + w]
                    )
                    fn_tiles[ct] = t

                # f_n chunk -> resident bf16 (phase-3 GEMM rhs AND transpose
                # source). GpSimd is otherwise idle; DVE takes the first
                # group's low tiles so the pipeline start isn't gated on it.
                for ct in range(CT):
                    r = fnbpool.tile([P, w], BF16, tag=f"fnb_{ct}_{gi}",
                                     name=f"fnb_{ct}_{gi}")
                    if gi < 2:
                        nc.vector.tensor_copy(r[:], fn_tiles[ct][:, 0:w])
                    else:
                        nc.scalar.activation(r[:], fn_tiles[ct][:, 0:w], CP)
                    for ch in grp:
                        fnb_by_ch[ct][ch] = (r, (ch - grp[0]) * 512)

                fmb_tiles = {}
                for ct in range(CT):
                    r = fmbpool.tile([P, 1024], BF16, tag=f"fmb{ct}")
                    if gi == 0 and ct < 2:
                        nc.vector.tensor_copy(r[:, 0:w], fm_tiles[ct][:, 0:w])
                    else:
                        nc.scalar.activation(r[:, 0:w], fm_tiles[ct][:, 0:w],
                                             CP)
                    fmb_tiles[ct] = r

                for ch in grp:
                    off = (ch - grp[0]) * 512
                    for su in range(4):
                        s = ch * 4 + su
                        sl = slice(off + su * P, off + (su + 1) * P)
                        # transpose f_m subchunk -> [n128, c512] (bf16)
                        tpm = work.tile([P, C], BF16, tag="wk0", name="tpm")
                        for ct in range(CT):
                            nc.tensor.transpose(
                                tpm[:, ct * P:(ct + 1) * P],
                                fmb_tiles[ct][:, sl], identb[:]
                            )
                        fmT = ftpool.tile([P, C], BF16, tag="fmT")
                        nc.vector.tensor_copy(fmT[:], tpm[:])

                        # transpose f_n subchunk (bf16)
                        tpn = work.tile([P, C], BF16, tag="wk1", name="tpn")
                        for ct in range(CT):
                            fnb, foff = fnb_by_ch[ct][ch]
                            nc.tensor.transpose(
                                tpn[:, ct * P:(ct + 1) * P],
                                fnb[:, foff + su * P:foff + (su + 1) * P],
                                identb[:]
                            )
                        fnT = ftpool.tile([P, C], BF16, tag="fnT")
                        nc.vector.tensor_copy(fnT[:], tpn[:])

                        # Gram: G[a-tile, :] += fmT[:, a-tile].T @ fnT
                        for at in range(CT):
                            nc.tensor.matmul(
                                g_ps[at][:],
                                fmT[:, at * P:(at + 1) * P],
                                fnT[:],
                                start=(s == 0),
                                stop=(s == NSUB - 1),
                            )

            # ---------- weights (needed from phase 2; loaded during phase 1
            # DMA slack; one wide DMA per matrix) ----------
            def load_bf16(dram, name):
                st = wstage.tile([P, CT, C], F32, tag="wst")
                nc.sync.dma_start(
                    st[:], dram.rearrange("(t p) c -> p t c", p=P)
                )
                res = []
                for rt in range(CT):
                    rs = wpool.tile([P, C], BF16, tag=f"{name}{rt}")
                    nc.scalar.activation(rs[:], st[:, rt, :], CP)
                    res.append(rs)
                return res

            WqT = load_bf16(wqt_d, "wqT")      # WqT[a][., (h,i)]
            WkT = load_bf16(wkt_d, "wkT")      # WkT[b][., (h,j)]
            WoutT = load_bf16(woutt_d, "woT")  # WoutT[e][., o]
            Wv_b = load_bf16(wv_d, "wv")       # Wv rows (h,j), cols c

            G_sb = []
            for at in range(CT):
                g = small.tile([P, C], BF16, tag=f"G{at}")
                if at < 2:
                    nc.vector.tensor_copy(g[:], g_ps[at][:])
                else:
                    nc.scalar.activation(g[:], g_ps[at][:], CP)
                G_sb.append(g)

            # ---------- phase 2: logits, softmax, value mixing ------------
            # T2T[b, (h,i)] = sum_a G[a, b] * WqT[a, (h,i)]
            # (G natural as stationary -> transposed product for free)
            T2T_sb = []
            for bt in range(CT):
                ps = work.tile([P, C], F32, tag="wk1", name="t2tps")
                for at in range(CT):
                    nc.tensor.matmul(
                        ps[:],
                        G_sb[at][:, bt * P:(bt + 1) * P],
                        WqT[at][:],
                        start=(at == 0),
                        stop=(at == CT - 1),
                    )
                t = small.tile([P, C], BF16, tag=f"T2T_{bt}")
                if bt < 2:
                    nc.vector.tensor_copy(t[:], ps[:])
                else:
                    nc.scalar.activation(t[:], ps[:], CP)
                T2T_sb.append(t)

            # Diagonal head-pair tiles of D^T = Wk @ T2T ; E^T = exp(scale*D^T)
            ET = []
            for jt in range(CT):
                sl = slice(jt * P, (jt + 1) * P)
                ps = work.tile([P, P], F32, tag="wk0", name="dps")
                for bt in range(CT):
                    nc.tensor.matmul(
                        ps[:], WkT[bt][:, sl], T2T_sb[bt][:, sl],
                        start=(bt == 0), stop=(bt == CT - 1),
                    )
                etmp = small.tile([P, P], BF16, tag="etmp")
                nc.scalar.activation(etmp[:], ps[:], EXP, scale=SCALE)
                e = small.tile([P, P], BF16, tag=f"G{jt}", name=f"ET{jt}")
                # zero the cross-head blocks so full-width matmuls (SE,
                # rowsums) see exact per-head separation
                nc.vector.tensor_mul(e[:], etmp[:], dmask[:])
                ET.append(e)

            # rowsums r[(h,i)] = sum_j E_h[i, j]
            inv_sb = []
            for it in range(CT):
                rps = work.tile([P, 2], F32, tag="wk1", name="rps")
                nc.tensor.matmul(rps[:], ET[it][:], ones2_b[:], start=True,
                                 stop=True)
                inv = small.tile([P, 1], F32, tag=f"inv{it}")
                nc.vector.reciprocal(inv[:], rps[:, 0:1])
                inv_sb.append(inv)

            # SE_h = E_h @ Wv_h ; S = SE * inv_r (deferred softmax division)
            S_sb = []
            for it in range(CT):
                seps = work.tile([P, C], F32, tag="wk0", name="seps")
                nc.tensor.matmul(
                    seps[:], ET[it][:], Wv_b[it][:], start=True, stop=True,
                )
                s_t = small.tile([P, C], BF16, tag=f"S{it}", name=f"S{it}")
                nc.vector.tensor_scalar_mul(s_t[:], seps[:], inv_sb[it][:])
                S_sb.append(s_t)

            # M^T[c, o] = sum_e S[e][:, c] * WoutT[e][:, o]
            MT_sb = []
            for ct in range(CT):
                ps = work.tile([P, C], F32, tag="wk1", name="mtps")
                for et in range(CT):
                    nc.tensor.matmul(
                        ps[:],
                        S_sb[et][:, ct * P:(ct + 1) * P],
                        WoutT[et][:],
                        start=(et == 0),
                        stop=(et == CT - 1),
                    )
                t = small.tile([P, C], BF16, tag=f"T2T_{ct}", name=f"MT{ct}")
                if ct < 2:
                    nc.vector.tensor_copy(t[:], ps[:])
                else:
                    nc.scalar.activation(t[:], ps[:], CP)
                MT_sb.append(t)

            # ---------- phase 3: out = M @ f_n + bout; store in ch-pairs ----
            opair = {}
            for ch in range(NCH):
                for ot in range(CT):
                    ps = gacc.tile([P, 512], F32, tag=f"g{ot}", name=f"ops{ot}")
                    for ct in range(CT):
                        fnb, foff = fnb_by_ch[ct][ch]
                        nc.tensor.matmul(
                            ps[:],
                            MT_sb[ct][:, ot * P:(ot + 1) * P],
                            fnb[:, foff:foff + 512],
                            start=(ct == 0),
                            stop=(ct == CT - 1),
                        )
                    paired = ch < 6
                    if paired and ch % 2 == 0:
                        opair[ot] = outst.tile([P, 1024], F32, tag=f"out{ot}",
                                               name=f"opair{ot}_{ch}")
                    elif not paired:
                        opair[ot] = outst.tile([P, 1024], F32, tag=f"out{ot}",
                                               name=f"osing{ot}_{ch}")
                    o = opair[ot]
                    hsl = slice((ch % 2) * 512, (ch % 2) * 512 + 512) \
                        if paired else slice(0, 512)
                    # ACT helps mid-stream; keep the last chunk all on DVE so
                    # the tail drains fast
                    if ot >= 2 and ch < NCH - 1:
                        nc.scalar.activation(o[:, hsl], ps[:], IDENT_FN,
                                             bias=bout_sb[ot])
                    else:
                        nc.vector.tensor_scalar_add(o[:, hsl], ps[:],
                                                    bout_sb[ot])
                    if paired and ch % 2 == 1:
                        nc.sync.dma_start(
                            out_d[ot * P:(ot + 1) * P,
                                  (ch - 1) * 512:(ch + 1) * 512],
                            o[:],
                        )
                    elif not paired:
                        nc.sync.dma_start(
                            out_d[ot * P:(ot + 1) * P,
                                  ch * 512:(ch + 1) * 512],
                            o[:, 0:512],
                        )

    nc.compile()
    return nc


def _get_nc():
    global _CACHED_NC
    if _CACHED_NC is None:
        _CACHED_NC = _build()
    return _CACHED_NC


def _get_runner():
    """Memoized PJRT runner: jax.jit-compiled once, reused across kernel()
    calls (run_bass_kernel_spmd rebuilds the jit closure every call, which
    forces a ~minute-long recompile)."""
    global _CACHED_RUNNER
    if _CACHED_RUNNER is not None:
        return _CACHED_RUNNER

    import jax
    from jax.sharding import Mesh, PartitionSpec
    from jax.experimental.shard_map import shard_map
    import concourse.mybir as mybir_
    from concourse.bass2jax import (
        _bass_exec_p,
        install_neuronx_cc_hook,
        partition_id_tensor,
    )

    nc = _get_nc()
    install_neuronx_cc_hook()

    partition_name = (
        nc.partition_id_tensor.name if nc.partition_id_tensor else None
    )
    in_names = []
    out_names = []
    out_avals = []
    out_shapes = []
    for alloc in nc.m.functions[0].allocations:
        if not isinstance(alloc, mybir_.MemoryLocationSet):
            continue
        name = alloc.memorylocations[0].name
        if alloc.kind == "ExternalInput":
            if name != partition_name:
                in_names.append(name)
        elif alloc.kind == "ExternalOutput":
            shape = tuple(alloc.tensor_shape)
            dtype = mybir_.dt.np(alloc.dtype)
            out_names.append(name)
            out_avals.append(jax.core.ShapedArray(shape, dtype))
            out_shapes.append((shape, dtype))
    n_params = len(in_names)
    n_outs = len(out_names)
    all_names = tuple(in_names + out_names)
    if partition_name is not None:
        all_names = all_names + (partition_name,)
    donate = tuple(range(n_params, n_params + n_outs))

    def _body(*args):
        operands = list(args)
        if partition_name is not None:
            operands.append(partition_id_tensor())
        outs = _bass_exec_p.bind(
            *operands,
            out_avals=tuple(out_avals),
            in_names=all_names,
            out_names=tuple(out_names),
            lowering_input_output_aliases=(),
            sim_require_finite=True,
            sim_require_nnan=True,
            nc=nc,
        )
        return tuple(outs)

    devices = jax.devices()[:B]
    mesh = Mesh(np.asarray(devices), ("core",))
    sharded = jax.jit(
        shard_map(
            _body,
            mesh=mesh,
            in_specs=(PartitionSpec("core"),) * (n_params + n_outs),
            out_specs=(PartitionSpec("core"),) * n_outs,
            check_rep=False,
        ),
        donate_argnums=donate,
        keep_unused=True,
    )

    def run(in_maps):
        concat_in = [
            np.concatenate([np.asarray(m[k]) for m in in_maps], axis=0)
            for k in in_names
        ]
        concat_zeros = [
            np.zeros((B * s[0], *s[1:]), dt) for (s, dt) in out_shapes
        ]
        out_arrs = sharded(*concat_in, *concat_zeros)
        return [
            {
                k: np.asarray(out_arrs[i]).reshape(B, *out_shapes[i][0])[c]
                for i, k in enumerate(out_names)
            }
            for c in range(B)
        ]

    _CACHED_RUNNER = run
    return run


def kernel(f_m, f_n, Wq, Wkv, Wout, bout, trace=False):
    f_m = np.ascontiguousarray(np.asarray(f_m, dtype=np.float32))
    f_n = np.ascontiguousarray(np.asarray(f_n, dtype=np.float32))
    Wq = np.ascontiguousarray(np.asarray(Wq, dtype=np.float32))
    Wkv = np.ascontiguousarray(np.asarray(Wkv, dtype=np.float32))
    Wout = np.ascontiguousarray(np.asarray(Wout, dtype=np.float32))
    bout = np.ascontiguousarray(np.asarray(bout, dtype=np.float32))

    b, c, h, w = f_m.shape
    nc = _get_nc()
    wqt = np.ascontiguousarray(Wq.T)
    wkt = np.ascontiguousarray(Wkv[:C].T)
    wv = np.ascontiguousarray(Wkv[C:])
    woutt = np.ascontiguousarray(Wout.T)
    cst = np.ascontiguousarray(
        np.concatenate(
            [_IDENT, _DMASK, bout.reshape(CT, P).T], axis=1
        ).astype(np.float32)
    )
    in_maps = [
        {
            "f_m": f_m[i].reshape(C, NN),
            "f_n": f_n[i].reshape(C, NN),
            "WqT": wqt,
            "WkT": wkt,
            "Wv": wv,
            "WoutT": woutt,
            "cst": cst,
        }
        for i in range(b)
    ]
    if trace:
        res = run_bass_kernel_spmd(
            nc, in_maps, core_ids=list(range(B)), trace=True
        )
        kernel.last_results = res
        results = res.results
    else:
        results = _get_runner()(in_maps)
    return np.stack([r["out"].reshape(c, h, w) for r in results])
